# revision 22
# baseline (speedup 1.0000x reference)
"""MCSPN Trainium2 kernel: guidance convs + fused softmax gates + 4-step CSPN.

Data-parallel over batch: 8 images -> 8 NeuronCores, one image per core.
fp16 everywhere (PSUM accum stays f32).

Per core:
  phase A: conv3x3 as 18 accumulating fp16 matmuls per row-pair, weight-cycled
           over groups of 2 row-pairs; feats tiles UNGUARDED/contiguous (4KB
           DMA packets); horizontal taps use edge-trimmed windows. Pipelined
           post-stages (one/two blocks behind the taps so no engine convoys):
           relu+BN (ACT) -> conv1x1 d-major (PE) -> exp (ACT) ->
           softmax fused in: partition-sum over the 4 directions via a
           tiled-eye matmul (PE) -> fast reciprocal + in-place normalize on
           the otherwise-idle DVE -> gate scatter DMAs issued from the idle
           GpSimd queue (keeps Sync free for feats loads).
  bridge:  pre-shift gu/gd gate planes along y (PE shift-matmul + ACT copy).
  phase B: packed h [128, K*W]; per step: 4 gate-product mults (DVE; GpSimd
           assists on the small k-chunk), 4 shift matmuls per 2-k chunk
           accumulating into PSUM (up/down via sub/super-diagonal, left/right
           via identity over shifted q windows), ACT evacuates PSUM -> nxt.
"""
import os
import sys

sys.path.insert(0, "/opt/trn_rl_repo")

import numpy as np

B, CIN, H, W = 8, 256, 128, 256
K = 19
MID = 128
KD = 4 * K  # 76
EPS = 1e-5
T_STEPS = 4
KW = K * W  # 4864
RG = 8      # feats rows per DMA group
WG = 2      # row-pairs per weight-cycle group


def _build():
    import concourse.bacc as bacc
    import concourse.mybir as mybir
    import concourse.tile as tile
    from concourse import bass

    f32 = mybir.dt.float32
    f16 = mybir.dt.float16
    Act = mybir.ActivationFunctionType
    Alu = mybir.AluOpType

    nc = bacc.Bacc("TRN2", target_bir_lowering=False)

    feats_d = nc.dram_tensor("feats", [CIN, H, W], f16, kind="ExternalInput")
    logits_d = nc.dram_tensor("logits", [H, K, W], f16, kind="ExternalInput")
    w1t_d = nc.dram_tensor("w1t", [128, 2, 9, MID], f16, kind="ExternalInput")
    bmid_d = nc.dram_tensor("bmid", [MID, 1], f32, kind="ExternalInput")
    w2t_d = nc.dram_tensor("w2t", [MID, KD], f16, kind="ExternalInput")
    b2_d = nc.dram_tensor("b2", [KD, 1], f32, kind="ExternalInput")
    sup_d = nc.dram_tensor("sup", [128, 128], f16, kind="ExternalInput")
    sdn_d = nc.dram_tensor("sdn", [128, 128], f16, kind="ExternalInput")
    idn_d = nc.dram_tensor("idn", [128, 128], f16, kind="ExternalInput")
    osum_d = nc.dram_tensor("osum", [KD, KD], f16, kind="ExternalInput")
    zer_d = nc.dram_tensor("zer", [1, KW], f16, kind="ExternalInput")
    out_d = nc.dram_tensor("out", [H, K, W], f16, kind="ExternalOutput")

    with tile.TileContext(nc) as tc:
        with tc.tile_pool(name="persist", bufs=1) as pp, \
             tc.tile_pool(name="hpool", bufs=1) as hp:
            e_all = pp.tile([128, 4 * KW], f16)       # gate planes, d-major
            h_a = hp.tile([128, KW], f16)
            h_b = hp.tile([128, KW], f16)
            w2c = pp.tile([MID, KD], f16)
            bmid = pp.tile([MID, 1], f32)
            b2c = pp.tile([KD, 1], f32)
            s_up = pp.tile([128, 128], f16)           # out[p] = v[p-1]
            s_dn = pp.tile([128, 128], f16)           # out[p] = v[p+1]
            iden = pp.tile([128, 128], f16)
            osum = pp.tile([KD, KD], f16)

            # ================= phase A: guidance =================
            with tc.tile_pool(name="w1p", bufs=1) as w1p:
                w1 = w1p.tile([128, 2, 9, MID], f16)

                with tc.tile_pool(name="frows", bufs=4) as frp, \
                     tc.tile_pool(name="xrow", bufs=6) as xrp, \
                     tc.tile_pool(name="estrip", bufs=8) as esp, \
                     tc.tile_pool(name="recip", bufs=4) as recp, \
                     tc.tile_pool(name="psA", bufs=6, space="PSUM") as psA, \
                     tc.tile_pool(name="psGS", bufs=2, space="PSUM") as psGS:
                    n_groups = H // RG
                    ftiles = {}

                    def load_group(g):
                        ft = frp.tile([128, 2, RG, W], f16, name=f"ft{g}",
                                      tag="ft")
                        for c in range(2):
                            nc.sync.dma_start(
                                out=ft[:, c],
                                in_=feats_d[c * 128:(c + 1) * 128,
                                            g * RG:(g + 1) * RG, :])
                        ftiles[g] = ft

                    emitted = 0

                    def ensure_groups(upto):
                        nonlocal emitted
                        while emitted < min(upto, n_groups):
                            load_group(emitted)
                            emitted += 1

                    # startup order: chunk-0 weights + group-0 feats first;
                    # everything else (not needed until later) after.
                    nc.sync.dma_start(out=w1[:, 0], in_=w1t_d[:, 0])
                    ensure_groups(1)
                    nc.sync.dma_start(out=w1[:, 1], in_=w1t_d[:, 1])

                    def load_consts():
                        nc.sync.dma_start(out=bmid[:], in_=bmid_d[:])
                        nc.sync.dma_start(out=b2c[:], in_=b2_d[:])
                        nc.sync.dma_start(out=w2c[:], in_=w2t_d[:])
                        nc.sync.dma_start(out=osum[:], in_=osum_d[:])
                        nc.sync.dma_start(out=s_up[:], in_=sup_d[:])
                        nc.sync.dma_start(out=s_dn[:], in_=sdn_d[:])
                        nc.sync.dma_start(out=iden[:], in_=idn_d[:])

                    # tap order: full-coverage ky=1 taps first & last so the
                    # start/stop matmuls cover every PSUM element; chunk-0
                    # taps lead so they only need the first w1 DMA.
                    taps = [(0, 1, 1)]
                    for c in range(2):
                        for ky in range(3):
                            for kx in range(3):
                                if (c, ky, kx) not in ((0, 1, 1), (1, 1, 1)):
                                    taps.append((c, ky, kx))
                    taps.append((1, 1, 1))

                    accs, xrs, accgs, ess = {}, {}, {}, {}

                    def emit_taps(wg):
                        for ti, (c, ky, kx) in enumerate(taps):
                            lw = w1[:, c, ky * 3 + kx, :]
                            first = ti == 0
                            last = ti == len(taps) - 1
                            for y in wg:
                                acc = accs[y]
                                rows = [(r, y + r + ky - 1) for r in range(2)
                                        if 0 <= y + r + ky - 1 < H]
                                mms = []
                                if (len(rows) == 2
                                        and rows[0][1] // RG == rows[1][1] // RG):
                                    g, ro = rows[0][1] // RG, rows[0][1] % RG
                                    mms.append((ftiles[g][:, c, ro:ro + 2, :],
                                                acc[:, 0:2, :]))
                                else:
                                    for (r, yin) in rows:
                                        g, ro = yin // RG, yin % RG
                                        mms.append((ftiles[g][:, c, ro, :],
                                                    acc[:, r, :]))
                                for rhs_full, oap in mms:
                                    if kx == 0:
                                        rhs = rhs_full[..., 0:W - 1]
                                        oap = oap[..., 1:W]
                                    elif kx == 2:
                                        rhs = rhs_full[..., 1:W]
                                        oap = oap[..., 0:W - 1]
                                    else:
                                        rhs = rhs_full
                                    nc.tensor.matmul(out=oap, lhsT=lw, rhs=rhs,
                                                     start=first, stop=last)

                    # edge rows never written by the shifted scatters: must
                    # be finite (0) or the 0-coeff NaN would poison shift MMs.
                    # (fp16 memset crashes walrus; DMA zeros instead)
                    nc.sync.dma_start(out=e_all[127:128, 2 * KW:3 * KW],
                                      in_=zer_d[:])
                    nc.sync.dma_start(out=e_all[0:1, 3 * KW:4 * KW],
                                      in_=zer_d[:])

                    def emit_relus(wg):
                        for y in wg:
                            xr = xrp.tile([MID, 2, W], f16, name="xr")
                            nc.scalar.activation(xr[:], accs[y][:], Act.Relu,
                                                 bias=bmid[:], scale=1.0)
                            xrs[y] = xr

                    def emit_c1(wg):
                        for y in wg:
                            accg = psGS.tile([KD, 2, W], f32, name="accg",
                                             tag="gs")
                            nc.tensor.matmul(out=accg[:], lhsT=w2c[:],
                                             rhs=xrs[y][:], start=True,
                                             stop=True)
                            accgs[y] = accg
                        for y in wg:
                            es = esp.tile([KD, 2, W], f16, name="es")
                            nc.scalar.activation(es[:], accgs[y][:], Act.Exp,
                                                 bias=b2c[:], scale=1.0)
                            ess[y] = es

                    def emit_post_b(wg):
                        # softmax normalization fused into phase A: direction
                        # sums via tiled-eye matmul, fast reciprocal +
                        # normalize on idle DVE. Scatters apply the gu/gd
                        # y-pre-shift for free: gu of row y lands at row y-1,
                        # gd at y+1 (edge rows dropped; consumers never read
                        # the dropped positions).
                        sps = {}
                        for y in wg:
                            sp = psGS.tile([KD, 2, W], f32, name="sum",
                                           tag="gs")
                            nc.tensor.matmul(out=sp[:], lhsT=osum[:],
                                             rhs=ess[y][:], start=True,
                                             stop=True)
                            sps[y] = sp
                        for y in wg:
                            rec = recp.tile([KD, 2, W], f32, name="rec")
                            nc.vector.reciprocal_approx_fast(out=rec[:],
                                                             in_=sps[y][:])
                            nc.vector.tensor_tensor(out=ess[y][:],
                                                    in0=ess[y][:], in1=rec[:],
                                                    op=Alu.mult)
                        for y in wg:
                            for r in range(2):
                                yy = y + r
                                nc.gpsimd.dma_start(
                                    out=e_all[yy:yy + 1, 0:2 * KW].rearrange(
                                        "p (c x) -> p c x", c=2 * K),
                                    in_=ess[y][0:2 * K, r, :])
                                if yy > 0:
                                    nc.gpsimd.dma_start(
                                        out=e_all[yy - 1:yy,
                                                  2 * KW:3 * KW].rearrange(
                                            "p (c x) -> p c x", c=K),
                                        in_=ess[y][2 * K:3 * K, r, :])
                                if yy < H - 1:
                                    nc.sync.dma_start(
                                        out=e_all[yy + 1:yy + 2,
                                                  3 * KW:4 * KW].rearrange(
                                            "p (c x) -> p c x", c=K),
                                        in_=ess[y][3 * K:4 * K, r, :])

                    # 2-block-lagged post pipeline: all ACT work emitted for a
                    # block (exp of wg i-2, relu of wg i-1) depends only on PE
                    # work from EARLIER blocks, so ACT runs bunched at block
                    # start and never builds a backlog that delays the psA
                    # WAR release for the next tap block.
                    pairs = list(range(0, H, 2))
                    wgs = [pairs[i:i + WG] for i in range(0, len(pairs), WG)]
                    for i, wg in enumerate(wgs):
                        if i > 1:
                            emit_c1(wgs[i - 2])
                        if i > 0:
                            emit_relus(wgs[i - 1])
                        ensure_groups((wg[-1] + 2) // RG + 2)
                        for y in wg:
                            accs[y] = psA.tile([MID, 2, W], f32,
                                               name=f"acc{y}", tag="acc")
                        emit_taps(wg)
                        if i > 2:
                            emit_post_b(wgs[i - 3])
                        if i == 1:
                            load_consts()
                        if i == 3:
                            # h0 load, placed away from the startup DMA burst
                            nc.sync.dma_start(out=h_a[:],
                                              in_=logits_d[:, :, :])
                    emit_c1(wgs[-2])
                    emit_relus(wgs[-1])
                    emit_post_b(wgs[-3])
                    emit_c1(wgs[-1])
                    emit_post_b(wgs[-2])
                    emit_post_b(wgs[-1])

            # ================= phase B: recurrence =================
            thirds = [(0, 8), (8, 16), (16, 19)]
            with tc.tile_pool(name="qp", bufs=1) as qp, \
                 tc.tile_pool(name="psB", bufs=2, space="PSUM") as psB:
                q_u = qp.tile([128, KW], f16)
                q_d = qp.tile([128, KW], f16)
                q_l = qp.tile([128, KW], f16)
                q_r = qp.tile([128, KW], f16)
                cur, nxt = h_a, h_b
                for t in range(T_STEPS):
                    for (k0, k1) in thirds:
                        nk = k1 - k0
                        f0, f1 = k0 * W, k1 * W
                        hseg = cur[:, f0:f1]
                        # gate products; gl/gr consumed via +-1 flat views
                        nc.vector.tensor_tensor(
                            out=q_u[:, f0:f1], in0=e_all[:, 2 * KW + f0:
                                                         2 * KW + f1],
                            in1=hseg, op=Alu.mult)
                        nc.vector.tensor_tensor(
                            out=q_d[:, f0:f1], in0=e_all[:, 3 * KW + f0:
                                                         3 * KW + f1],
                            in1=hseg, op=Alu.mult)
                        nc.vector.tensor_tensor(
                            out=q_l[:, f0:f1], in0=e_all[:, f0 + 1:f1 + 1],
                            in1=hseg, op=Alu.mult)
                        nc.vector.tensor_tensor(
                            out=q_r[:, f0:f1], in0=e_all[:, KW + f0 - 1:
                                                         KW + f1 - 1],
                            in1=hseg, op=Alu.mult)
                        ps = psB.tile([128, 8, W], f32, name="ps")
                        chunks = [(a, min(a + 2, nk)) for a in range(0, nk, 2)]
                        for (a, b) in chunks:
                            nc.tensor.matmul(
                                out=ps[:, a:b, :], lhsT=s_up[:],
                                rhs=q_u[:, f0 + a * W:f0 + b * W],
                                start=True, stop=False)
                        for (a, b) in chunks:
                            nc.tensor.matmul(
                                out=ps[:, a:b, :], lhsT=s_dn[:],
                                rhs=q_d[:, f0 + a * W:f0 + b * W],
                                start=False, stop=False)
                        for (a, b) in chunks:
                            # agg[x] += q_l[x-1] for x>=1
                            nc.tensor.matmul(
                                out=ps[:, a:b, 1:W], lhsT=iden[:],
                                rhs=q_l[:, f0 + a * W:f0 + b * W].rearrange(
                                    "p (k x) -> p k x", k=b - a)[:, :, 0:W - 1],
                                start=False, stop=False)
                        for (a, b) in chunks:
                            # agg[x] += q_r[x+1] for x<W-1
                            nc.tensor.matmul(
                                out=ps[:, a:b, 0:W - 1], lhsT=iden[:],
                                rhs=q_r[:, f0 + a * W:f0 + b * W].rearrange(
                                    "p (k x) -> p k x", k=b - a)[:, :, 1:W],
                                start=False, stop=True)
                        nc.scalar.activation(nxt[:, f0:f1], ps[:, 0:nk, :],
                                             Act.Copy)
                    cur, nxt = nxt, cur

                nc.sync.dma_start(out=out_d[:, :, :],
                                  in_=cur[:].rearrange("p (k x) -> p k x", k=K))

    nc.compile()
    return nc


_NC_CACHE = None


def kernel(feats, logits, w1, gamma, beta, mean, var, w2, b2):
    global _NC_CACHE
    from concourse.bass_utils import run_bass_kernel_spmd

    feats = np.asarray(feats, dtype=np.float32)
    logits = np.asarray(logits, dtype=np.float32)
    w1 = np.asarray(w1, dtype=np.float32)
    w2 = np.asarray(w2, dtype=np.float32)
    b2 = np.asarray(b2, dtype=np.float32)
    gamma = np.asarray(gamma, dtype=np.float32)
    beta = np.asarray(beta, dtype=np.float32)
    mean = np.asarray(mean, dtype=np.float32)
    var = np.asarray(var, dtype=np.float32)

    inv = gamma / np.sqrt(var + EPS)
    w1f = (w1 * inv[:, None, None, None]).astype(np.float32)  # [MID,CIN,3,3]
    bmid = (beta - mean * inv).astype(np.float32)[:, None]    # [MID,1]
    # [cin_in_chunk 128, chunk 2, tap 9, mid 128]
    w1t = (w1f.transpose(1, 2, 3, 0)                  # [CIN,3,3,MID]
           .reshape(2, 128, 9, MID)
           .transpose(1, 0, 2, 3)).astype(np.float16).copy()
    # d-major output channel order: new channel p = d*K + k <- old k*4 + d
    perm = np.array([k * 4 + d for d in range(4) for k in range(K)])
    w2t = w2.reshape(KD, MID)[perm].T.astype(np.float16).copy()  # [MID,KD]
    b2c = b2[perm][:, None].astype(np.float32).copy()
    s_up = np.eye(128, k=1, dtype=np.float16)         # out[m]=v[m-1]
    s_dn = np.eye(128, k=-1, dtype=np.float16)        # out[m]=v[m+1]
    idn = np.eye(128, dtype=np.float16)
    osum = np.tile(np.eye(K, dtype=np.float16), (4, 4))  # [KD,KD] dir-sum

    if _NC_CACHE is None:
        _NC_CACHE = _build()
    nc = _NC_CACHE

    in_maps = []
    for i in range(B):
        in_maps.append({
            "feats": np.ascontiguousarray(feats[i]).astype(np.float16),
            "logits": np.ascontiguousarray(
                logits[i].transpose(1, 0, 2)).astype(np.float16),
            "w1t": w1t, "bmid": bmid, "w2t": w2t, "b2": b2c,
            "sup": s_up, "sdn": s_dn, "idn": idn, "osum": osum,
            "zer": np.zeros((1, KW), np.float16),
        })

    trace = bool(os.environ.get("KTRACE"))
    res = run_bass_kernel_spmd(nc, in_maps, list(range(B)), trace=trace)
    if trace and res.exec_time_ns is not None:
        print(f"HW exec time: {res.exec_time_ns} ns")
    out = np.stack([res.results[i]["out"] for i in range(B)], axis=0)
    return out.transpose(0, 2, 1, 3).astype(np.float32)


if __name__ == "__main__":
    rng = np.random.default_rng(0)
    ins = {
        "feats": rng.standard_normal((B, CIN, H, W), dtype=np.float32),
        "logits": rng.standard_normal((B, K, H, W), dtype=np.float32),
        "w1": rng.standard_normal((MID, CIN, 3, 3), dtype=np.float32) / 48.0,
        "gamma": rng.standard_normal(MID).astype(np.float32) * 0.1 + 1.0,
        "beta": rng.standard_normal(MID).astype(np.float32) * 0.1,
        "mean": rng.standard_normal(MID).astype(np.float32) * 0.1,
        "var": rng.random(MID).astype(np.float32) + 0.5,
        "w2": rng.standard_normal((KD, MID, 1, 1)).astype(np.float32) / 11.3,
        "b2": rng.standard_normal(KD).astype(np.float32) * 0.01,
    }
    o = kernel(**ins)
    print("kernel out", o.shape, o.dtype, np.abs(o).mean())


# revision 24
# speedup vs baseline: 1.2369x; 1.2369x over previous
"""MCSPN Trainium2 kernel: guidance convs + fused softmax gates + 4-step CSPN.

Data-parallel over batch: 8 images -> 8 NeuronCores, one image per core.
fp16 everywhere (PSUM accum stays f32).

Per core:
  phase A: conv3x3 as 18 accumulating fp16 matmuls per row-pair, weight-cycled
           over groups of 2 row-pairs; feats tiles UNGUARDED/contiguous (4KB
           DMA packets); horizontal taps use edge-trimmed windows. Pipelined
           post-stages (one/two blocks behind the taps so no engine convoys):
           relu+BN (ACT) -> conv1x1 d-major (PE) -> exp (ACT) ->
           softmax fused in: partition-sum over the 4 directions via a
           tiled-eye matmul (PE) -> fast reciprocal + in-place normalize on
           the otherwise-idle DVE -> gate scatter DMAs issued from the idle
           GpSimd queue (keeps Sync free for feats loads).
  bridge:  pre-shift gu/gd gate planes along y (PE shift-matmul + ACT copy).
  phase B: packed h [128, K*W]; per step: 4 gate-product mults (DVE; GpSimd
           assists on the small k-chunk), 4 shift matmuls per 2-k chunk
           accumulating into PSUM (up/down via sub/super-diagonal, left/right
           via identity over shifted q windows), ACT evacuates PSUM -> nxt.
"""
import os
import sys

sys.path.insert(0, "/opt/trn_rl_repo")

import numpy as np

B, CIN, H, W = 8, 256, 128, 256
K = 19
MID = 128
KD = 4 * K  # 76
EPS = 1e-5
T_STEPS = 4
KW = K * W  # 4864
RG = 8      # feats rows per DMA group
WG = 2      # row-pairs per weight-cycle group


def _build():
    import concourse.bacc as bacc
    import concourse.mybir as mybir
    import concourse.tile as tile
    from concourse import bass

    f32 = mybir.dt.float32
    f16 = mybir.dt.float16
    Act = mybir.ActivationFunctionType
    Alu = mybir.AluOpType

    nc = bacc.Bacc("TRN2", target_bir_lowering=False)

    feats_d = nc.dram_tensor("feats", [CIN, H, W], f16, kind="ExternalInput")
    logits_d = nc.dram_tensor("logits", [H, K, W], f16, kind="ExternalInput")
    w1t_d = nc.dram_tensor("w1t", [128, 2, 9, MID], f16, kind="ExternalInput")
    bmid_d = nc.dram_tensor("bmid", [MID, 1], f32, kind="ExternalInput")
    w2t_d = nc.dram_tensor("w2t", [MID, KD], f16, kind="ExternalInput")
    b2_d = nc.dram_tensor("b2", [KD, 1], f32, kind="ExternalInput")
    sup_d = nc.dram_tensor("sup", [128, 128], f16, kind="ExternalInput")
    sdn_d = nc.dram_tensor("sdn", [128, 128], f16, kind="ExternalInput")
    idn_d = nc.dram_tensor("idn", [128, 128], f16, kind="ExternalInput")
    osum_d = nc.dram_tensor("osum", [KD, KD], f16, kind="ExternalInput")
    zer_d = nc.dram_tensor("zer", [1, KW], f16, kind="ExternalInput")
    out_d = nc.dram_tensor("out", [H, K, W], f16, kind="ExternalOutput")

    with tile.TileContext(nc) as tc:
        with tc.tile_pool(name="persist", bufs=1) as pp, \
             tc.tile_pool(name="hpool", bufs=1) as hp:
            e_all = pp.tile([128, 4 * KW], f16)       # gate planes, d-major
            h_a = hp.tile([128, KW], f16)
            h_b = hp.tile([128, KW], f16)
            w2c = pp.tile([MID, KD], f16)
            bmid = pp.tile([MID, 1], f32)
            b2c = pp.tile([KD, 1], f32)
            s_up = pp.tile([128, 128], f16)           # out[p] = v[p-1]
            s_dn = pp.tile([128, 128], f16)           # out[p] = v[p+1]
            iden = pp.tile([128, 128], f16)
            osum = pp.tile([KD, KD], f16)

            # ================= phase A: guidance =================
            with tc.tile_pool(name="w1p", bufs=1) as w1p:
                w1 = w1p.tile([128, 2, 9, MID], f16)

                with tc.tile_pool(name="frows", bufs=4) as frp, \
                     tc.tile_pool(name="xrow", bufs=6) as xrp, \
                     tc.tile_pool(name="estrip", bufs=8) as esp, \
                     tc.tile_pool(name="recip", bufs=4) as recp, \
                     tc.tile_pool(name="psA", bufs=4, space="PSUM") as psA, \
                     tc.tile_pool(name="psG", bufs=2, space="PSUM") as psG, \
                     tc.tile_pool(name="psS", bufs=2, space="PSUM") as psS:
                    n_groups = H // RG
                    ftiles = {}

                    def load_group(g):
                        ft = frp.tile([128, 2, RG, W], f16, name=f"ft{g}",
                                      tag="ft")
                        for c in range(2):
                            nc.sync.dma_start(
                                out=ft[:, c],
                                in_=feats_d[c * 128:(c + 1) * 128,
                                            g * RG:(g + 1) * RG, :])
                        ftiles[g] = ft

                    emitted = 0

                    def ensure_groups(upto):
                        nonlocal emitted
                        while emitted < min(upto, n_groups):
                            load_group(emitted)
                            emitted += 1

                    # startup order: chunk-0 weights + group-0 feats first;
                    # everything else (not needed until later) after.
                    nc.sync.dma_start(out=w1[:, 0], in_=w1t_d[:, 0])
                    ensure_groups(1)
                    nc.sync.dma_start(out=w1[:, 1], in_=w1t_d[:, 1])
                    # tiny, and consumed by instructions emitted at block 1:
                    # must be emitted before any consumer
                    nc.sync.dma_start(out=bmid[:], in_=bmid_d[:])
                    nc.sync.dma_start(out=b2c[:], in_=b2_d[:])

                    def load_consts():
                        nc.sync.dma_start(out=w2c[:], in_=w2t_d[:])
                        nc.sync.dma_start(out=osum[:], in_=osum_d[:])
                        nc.sync.dma_start(out=s_up[:], in_=sup_d[:])
                        nc.sync.dma_start(out=s_dn[:], in_=sdn_d[:])
                        nc.sync.dma_start(out=iden[:], in_=idn_d[:])

                    # tap order: full-coverage ky=1 taps first & last so the
                    # start/stop matmuls cover every PSUM element; chunk-0
                    # taps lead so they only need the first w1 DMA.
                    taps = [(0, 1, 1)]
                    for c in range(2):
                        for ky in range(3):
                            for kx in range(3):
                                if (c, ky, kx) not in ((0, 1, 1), (1, 1, 1)):
                                    taps.append((c, ky, kx))
                    taps.append((1, 1, 1))

                    accs, xrs, accgs, ess = {}, {}, {}, {}

                    def emit_taps(wg):
                        for y in wg:
                            for ti, (c, ky, kx) in enumerate(taps):
                                lw = w1[:, c, ky * 3 + kx, :]
                                first = ti == 0
                                last = ti == len(taps) - 1
                                acc = accs[y]
                                rows = [(r, y + r + ky - 1) for r in range(2)
                                        if 0 <= y + r + ky - 1 < H]
                                mms = []
                                if (len(rows) == 2
                                        and rows[0][1] // RG == rows[1][1] // RG):
                                    g, ro = rows[0][1] // RG, rows[0][1] % RG
                                    mms.append((ftiles[g][:, c, ro:ro + 2, :],
                                                acc[:, 0:2, :]))
                                else:
                                    for (r, yin) in rows:
                                        g, ro = yin // RG, yin % RG
                                        mms.append((ftiles[g][:, c, ro, :],
                                                    acc[:, r, :]))
                                for rhs_full, oap in mms:
                                    if kx == 0:
                                        rhs = rhs_full[..., 0:W - 1]
                                        oap = oap[..., 1:W]
                                    elif kx == 2:
                                        rhs = rhs_full[..., 1:W]
                                        oap = oap[..., 0:W - 1]
                                    else:
                                        rhs = rhs_full
                                    nc.tensor.matmul(out=oap, lhsT=lw, rhs=rhs,
                                                     start=first, stop=last)

                    # edge rows never written by the shifted scatters: must
                    # be finite (0) or the 0-coeff NaN would poison shift MMs.
                    # (fp16 memset crashes walrus; DMA zeros instead)
                    nc.sync.dma_start(out=e_all[127:128, 2 * KW:3 * KW],
                                      in_=zer_d[:])
                    nc.sync.dma_start(out=e_all[0:1, 3 * KW:4 * KW],
                                      in_=zer_d[:])

                    def emit_relus(wg):
                        for y in wg:
                            xr = xrp.tile([MID, 2, W], f16, name="xr")
                            nc.scalar.activation(xr[:], accs[y][:], Act.Relu,
                                                 bias=bmid[:], scale=1.0)
                            xrs[y] = xr

                    def emit_c1(wg):
                        for y in wg:
                            accg = psG.tile([KD, 2, W], f32, name="accg")
                            nc.tensor.matmul(out=accg[:], lhsT=w2c[:],
                                             rhs=xrs[y][:], start=True,
                                             stop=True)
                            accgs[y] = accg
                        for y in wg:
                            es = esp.tile([KD, 2, W], f16, name="es")
                            nc.scalar.activation(es[:], accgs[y][:], Act.Exp,
                                                 bias=b2c[:], scale=1.0)
                            ess[y] = es

                    def emit_post_b(wg):
                        # softmax normalization fused into phase A: direction
                        # sums via tiled-eye matmul, fast reciprocal +
                        # normalize on idle DVE. Scatters apply the gu/gd
                        # y-pre-shift for free: gu of row y lands at row y-1,
                        # gd at y+1 (edge rows dropped; consumers never read
                        # the dropped positions).
                        sps = {}
                        for y in wg:
                            sp = psS.tile([KD, 2, W], f32, name="sum")
                            nc.tensor.matmul(out=sp[:], lhsT=osum[:],
                                             rhs=ess[y][:], start=True,
                                             stop=True)
                            sps[y] = sp
                        for y in wg:
                            rec = recp.tile([KD, 2, W], f32, name="rec")
                            nc.vector.reciprocal_approx_fast(out=rec[:],
                                                             in_=sps[y][:])
                            nc.vector.tensor_tensor(out=ess[y][:],
                                                    in0=ess[y][:], in1=rec[:],
                                                    op=Alu.mult)
                        for y in wg:
                            for r in range(2):
                                yy = y + r
                                nc.gpsimd.dma_start(
                                    out=e_all[yy:yy + 1, 0:2 * KW].rearrange(
                                        "p (c x) -> p c x", c=2 * K),
                                    in_=ess[y][0:2 * K, r, :])
                                if yy > 0:
                                    nc.gpsimd.dma_start(
                                        out=e_all[yy - 1:yy,
                                                  2 * KW:3 * KW].rearrange(
                                            "p (c x) -> p c x", c=K),
                                        in_=ess[y][2 * K:3 * K, r, :])
                                if yy < H - 1:
                                    nc.sync.dma_start(
                                        out=e_all[yy + 1:yy + 2,
                                                  3 * KW:4 * KW].rearrange(
                                            "p (c x) -> p c x", c=K),
                                        in_=ess[y][3 * K:4 * K, r, :])

                    # 2-block-lagged post pipeline: all ACT work emitted for a
                    # block (exp of wg i-2, relu of wg i-1) depends only on PE
                    # work from EARLIER blocks, so ACT runs bunched at block
                    # start and never builds a backlog that delays the psA
                    # WAR release for the next tap block.
                    pairs = list(range(0, H, 2))
                    wgs = [pairs[i:i + WG] for i in range(0, len(pairs), WG)]
                    for i, wg in enumerate(wgs):
                        if i > 1:
                            emit_c1(wgs[i - 2])
                        if i > 0:
                            emit_relus(wgs[i - 1])
                        ensure_groups((wg[-1] + 2) // RG + 2)
                        for y in wg:
                            accs[y] = psA.tile([MID, 2, W], f32,
                                               name=f"acc{y}", tag="acc")
                        emit_taps(wg)
                        if i > 2:
                            emit_post_b(wgs[i - 3])
                        if i == 1:
                            load_consts()
                        if i == 3:
                            # h0 load, placed away from the startup DMA burst
                            nc.sync.dma_start(out=h_a[:],
                                              in_=logits_d[:, :, :])
                    emit_c1(wgs[-2])
                    emit_relus(wgs[-1])
                    emit_post_b(wgs[-3])
                    emit_c1(wgs[-1])
                    emit_post_b(wgs[-2])
                    emit_post_b(wgs[-1])

            # ================= phase B: recurrence =================
            thirds = [(0, 8), (8, 16), (16, 19)]
            with tc.tile_pool(name="qp", bufs=1) as qp, \
                 tc.tile_pool(name="psB", bufs=2, space="PSUM") as psB:
                q_u = qp.tile([128, KW], f16)
                q_d = qp.tile([128, KW], f16)
                q_l = qp.tile([128, KW], f16)
                q_r = qp.tile([128, KW], f16)
                cur, nxt = h_a, h_b
                for t in range(T_STEPS):
                    for (k0, k1) in thirds:
                        nk = k1 - k0
                        f0, f1 = k0 * W, k1 * W
                        hseg = cur[:, f0:f1]
                        # gate products; gl/gr consumed via +-1 flat views
                        nc.vector.tensor_tensor(
                            out=q_u[:, f0:f1], in0=e_all[:, 2 * KW + f0:
                                                         2 * KW + f1],
                            in1=hseg, op=Alu.mult)
                        nc.vector.tensor_tensor(
                            out=q_d[:, f0:f1], in0=e_all[:, 3 * KW + f0:
                                                         3 * KW + f1],
                            in1=hseg, op=Alu.mult)
                        nc.vector.tensor_tensor(
                            out=q_l[:, f0:f1], in0=e_all[:, f0 + 1:f1 + 1],
                            in1=hseg, op=Alu.mult)
                        nc.vector.tensor_tensor(
                            out=q_r[:, f0:f1], in0=e_all[:, KW + f0 - 1:
                                                         KW + f1 - 1],
                            in1=hseg, op=Alu.mult)
                        ps = psB.tile([128, 8, W], f32, name="ps")
                        chunks = [(a, min(a + 2, nk)) for a in range(0, nk, 2)]
                        for (a, b) in chunks:
                            nc.tensor.matmul(
                                out=ps[:, a:b, :], lhsT=s_up[:],
                                rhs=q_u[:, f0 + a * W:f0 + b * W],
                                start=True, stop=False)
                        for (a, b) in chunks:
                            nc.tensor.matmul(
                                out=ps[:, a:b, :], lhsT=s_dn[:],
                                rhs=q_d[:, f0 + a * W:f0 + b * W],
                                start=False, stop=False)
                        for (a, b) in chunks:
                            # agg[x] += q_l[x-1] for x>=1
                            nc.tensor.matmul(
                                out=ps[:, a:b, 1:W], lhsT=iden[:],
                                rhs=q_l[:, f0 + a * W:f0 + b * W].rearrange(
                                    "p (k x) -> p k x", k=b - a)[:, :, 0:W - 1],
                                start=False, stop=False)
                        for (a, b) in chunks:
                            # agg[x] += q_r[x+1] for x<W-1
                            nc.tensor.matmul(
                                out=ps[:, a:b, 0:W - 1], lhsT=iden[:],
                                rhs=q_r[:, f0 + a * W:f0 + b * W].rearrange(
                                    "p (k x) -> p k x", k=b - a)[:, :, 1:W],
                                start=False, stop=True)
                        nc.scalar.activation(nxt[:, f0:f1], ps[:, 0:nk, :],
                                             Act.Copy)
                    cur, nxt = nxt, cur

                nc.sync.dma_start(out=out_d[:, :, :],
                                  in_=cur[:].rearrange("p (k x) -> p k x", k=K))

    nc.compile()
    return nc


_NC_CACHE = None


def kernel(feats, logits, w1, gamma, beta, mean, var, w2, b2):
    global _NC_CACHE
    from concourse.bass_utils import run_bass_kernel_spmd

    feats = np.asarray(feats, dtype=np.float32)
    logits = np.asarray(logits, dtype=np.float32)
    w1 = np.asarray(w1, dtype=np.float32)
    w2 = np.asarray(w2, dtype=np.float32)
    b2 = np.asarray(b2, dtype=np.float32)
    gamma = np.asarray(gamma, dtype=np.float32)
    beta = np.asarray(beta, dtype=np.float32)
    mean = np.asarray(mean, dtype=np.float32)
    var = np.asarray(var, dtype=np.float32)

    inv = gamma / np.sqrt(var + EPS)
    w1f = (w1 * inv[:, None, None, None]).astype(np.float32)  # [MID,CIN,3,3]
    bmid = (beta - mean * inv).astype(np.float32)[:, None]    # [MID,1]
    # [cin_in_chunk 128, chunk 2, tap 9, mid 128]
    w1t = (w1f.transpose(1, 2, 3, 0)                  # [CIN,3,3,MID]
           .reshape(2, 128, 9, MID)
           .transpose(1, 0, 2, 3)).astype(np.float16).copy()
    # d-major output channel order: new channel p = d*K + k <- old k*4 + d
    perm = np.array([k * 4 + d for d in range(4) for k in range(K)])
    w2t = w2.reshape(KD, MID)[perm].T.astype(np.float16).copy()  # [MID,KD]
    b2c = b2[perm][:, None].astype(np.float32).copy()
    s_up = np.eye(128, k=1, dtype=np.float16)         # out[m]=v[m-1]
    s_dn = np.eye(128, k=-1, dtype=np.float16)        # out[m]=v[m+1]
    idn = np.eye(128, dtype=np.float16)
    osum = np.tile(np.eye(K, dtype=np.float16), (4, 4))  # [KD,KD] dir-sum

    if _NC_CACHE is None:
        _NC_CACHE = _build()
    nc = _NC_CACHE

    in_maps = []
    for i in range(B):
        in_maps.append({
            "feats": np.ascontiguousarray(feats[i]).astype(np.float16),
            "logits": np.ascontiguousarray(
                logits[i].transpose(1, 0, 2)).astype(np.float16),
            "w1t": w1t, "bmid": bmid, "w2t": w2t, "b2": b2c,
            "sup": s_up, "sdn": s_dn, "idn": idn, "osum": osum,
            "zer": np.zeros((1, KW), np.float16),
        })

    trace = bool(os.environ.get("KTRACE"))
    res = run_bass_kernel_spmd(nc, in_maps, list(range(B)), trace=trace)
    if trace and res.exec_time_ns is not None:
        print(f"HW exec time: {res.exec_time_ns} ns")
    out = np.stack([res.results[i]["out"] for i in range(B)], axis=0)
    return out.transpose(0, 2, 1, 3).astype(np.float32)


if __name__ == "__main__":
    rng = np.random.default_rng(0)
    ins = {
        "feats": rng.standard_normal((B, CIN, H, W), dtype=np.float32),
        "logits": rng.standard_normal((B, K, H, W), dtype=np.float32),
        "w1": rng.standard_normal((MID, CIN, 3, 3), dtype=np.float32) / 48.0,
        "gamma": rng.standard_normal(MID).astype(np.float32) * 0.1 + 1.0,
        "beta": rng.standard_normal(MID).astype(np.float32) * 0.1,
        "mean": rng.standard_normal(MID).astype(np.float32) * 0.1,
        "var": rng.random(MID).astype(np.float32) + 0.5,
        "w2": rng.standard_normal((KD, MID, 1, 1)).astype(np.float32) / 11.3,
        "b2": rng.standard_normal(KD).astype(np.float32) * 0.01,
    }
    o = kernel(**ins)
    print("kernel out", o.shape, o.dtype, np.abs(o).mean())
